# revision 1
# baseline (speedup 1.0000x reference)
"""Cross-attention (B=2, Q=1024, N=4096, C=768, H=12, D=64) with bilinearly
interpolated relative position bias, on 8 Trainium2 NeuronCores.

Sharding: core c handles batch b = c//4 and heads 3*(c%4) .. 3*(c%4)+2
(tensor-parallel over heads, data-parallel over batch). Each core computes its
heads' contribution to out[b] @ Wo (pre-bias); the host sums the four partial
results per batch and adds bo.

Device algorithm per core (all matmuls float32r, moving dim >= 256):
  qbT[h]  = (Wq_h^T @ q^T) * scale + bq          [64, 1024]   (d-major)
  kbT[h]  = Wk_h^T @ kv^T + bk                   [64, 4096]
  vb[n]   = kv @ Wv_h + bv                       [4096, 64]   (n-major)
  S^T     = [kbT; Wn]^T-contraction [qbT; B1T]   K=96 fuses the interpolated
            bias: bias[h,q,n] = sum_j B1[h,q,j] * Wn[j,n]
  P^T     = exp(S^T)            (no max-subtraction; logits are O(1))
  O^T[h]  = [vb_h | 1]^T @ P^T                   [65, 512]  row 64 = denominator
  out    += (O^T[h]/denom)^T-contraction Wo_h    [1024, 768]
"""

import numpy as np

B, Q, N, C = 2, 1024, 4096, 768
H, D, REL = 12, 64, 32
SCALE = 1.0 / np.sqrt(D)
HPC = 3            # heads per core
N_CORES = 8
NCH = 8            # 512-wide n-chunks
F32 = None         # filled lazily (mybir import)
F32R = None

_COMPILED = None   # (nc,) cached across kernel() calls


def _lin_coords(n_out, n_in):
    pos = np.arange(n_out, dtype=np.float32) * np.float32((n_in - 1) / (n_out - 1))
    lo = np.clip(np.floor(pos).astype(np.int32), 0, n_in - 1)
    hi = np.clip(lo + 1, 0, n_in - 1)
    w = (pos - lo.astype(np.float32)).astype(np.float32)
    return lo, hi, w


def _host_bias_parts(rel_pos_bias):
    """B1: [H, Q, 32] q-interpolated bias; Wn: [32, N] n-interp weights."""
    lq, hq, wq = _lin_coords(Q, REL)
    ln, hn, wn = _lin_coords(N, REL)
    b1 = (rel_pos_bias[:, lq, :] * (1.0 - wq)[None, :, None]
          + rel_pos_bias[:, hq, :] * wq[None, :, None]).astype(np.float32)
    w_n = np.zeros((REL, N), np.float32)
    np.add.at(w_n, (ln, np.arange(N)), (1.0 - wn))
    np.add.at(w_n, (hn, np.arange(N)), wn)
    return b1, w_n


def _build():
    import concourse.tile as tile
    from concourse import bacc, mybir
    import concourse.bass as bass

    F32 = mybir.dt.float32
    F32R = mybir.dt.float32r
    KT = 6  # C // 128 contraction tiles

    nc = bacc.Bacc("TRN2", target_bir_lowering=False, debug=False,
                   enable_asserts=False, num_devices=N_CORES)

    qT = nc.dram_tensor("qT", [C, Q], F32R, kind="ExternalInput")
    kvT = nc.dram_tensor("kvT", [C, N], F32R, kind="ExternalInput")
    wq = nc.dram_tensor("wq", [C, 192], F32R, kind="ExternalInput")    # pre-scaled
    wk = nc.dram_tensor("wk", [C, 192], F32R, kind="ExternalInput")
    wv = nc.dram_tensor("wv", [C, 256], F32R, kind="ExternalInput")    # zero-padded
    wo = nc.dram_tensor("wo", [HPC, D, C], F32R, kind="ExternalInput")
    bqs = nc.dram_tensor("bqs", [D, HPC], F32, kind="ExternalInput")   # pre-scaled
    bks = nc.dram_tensor("bks", [D, HPC], F32, kind="ExternalInput")
    bvb = nc.dram_tensor("bvb", [128, 192], F32, kind="ExternalInput")  # bcast bv
    b1t = nc.dram_tensor("b1t", [HPC, REL, Q], F32R, kind="ExternalInput")
    w_n = nc.dram_tensor("w_n", [REL, N], F32R, kind="ExternalInput")
    ones3 = nc.dram_tensor("ones3", [128, HPC], F32R, kind="ExternalInput")
    out_p = nc.dram_tensor("out_p", [Q, C], F32, kind="ExternalOutput")

    EXP = mybir.ActivationFunctionType.Exp
    MUL = mybir.AluOpType.mult
    ADD = mybir.AluOpType.add

    with tile.TileContext(nc) as tc:
        with (
            tc.tile_pool(name="wpool", bufs=1) as wpool,
            tc.tile_pool(name="persist", bufs=1) as pers,
            tc.tile_pool(name="stream", bufs=2) as stream,
            tc.tile_pool(name="psB", bufs=2, space="PSUM") as psB,
        ):
            # ---- constants / weights ----
            wq_sb = wpool.tile([128, KT, 192], F32R, name="wq_sb")
            nc.sync.dma_start(out=wq_sb, in_=wq.rearrange("(t p) m -> p t m", p=128))
            wk_sb = wpool.tile([128, KT, 192], F32R, name="wk_sb")
            nc.sync.dma_start(out=wk_sb, in_=wk.rearrange("(t p) m -> p t m", p=128))
            wv_sb = wpool.tile([128, KT, 256], F32R, name="wv_sb")
            nc.sync.dma_start(out=wv_sb, in_=wv.rearrange("(t p) m -> p t m", p=128))
            wo_sb = wpool.tile([D, HPC, C], F32R, name="wo_sb")
            nc.sync.dma_start(out=wo_sb, in_=wo.rearrange("h p n -> p h n"))
            bqs_sb = wpool.tile([D, HPC], F32, name="bqs_sb")
            nc.sync.dma_start(out=bqs_sb, in_=bqs[:, :])
            bks_sb = wpool.tile([D, HPC], F32, name="bks_sb")
            nc.sync.dma_start(out=bks_sb, in_=bks[:, :])
            bvb_sb = wpool.tile([128, 192], F32, name="bvb_sb")
            nc.sync.dma_start(out=bvb_sb, in_=bvb[:, :])

            # ---- persistent per-head / per-chunk tiles ----
            qTp = [pers.tile([96, Q], F32R, name=f"qTp{h}", tag=f"qTp{h}")
                   for h in range(HPC)]
            kbTp = [[pers.tile([96, 512], F32R, name=f"kbT{h}_{ch}",
                               tag=f"kbT{h}_{ch}")
                     for ch in range(NCH)] for h in range(HPC)]
            vb = [pers.tile([128, 195], F32R, name=f"vb{s}", tag=f"vb{s}")
                  for s in range(N // 128)]

            # ---- phase A: q projection ----
            with tc.tile_pool(name="qload", bufs=1) as qload, \
                 tc.tile_pool(name="psA0", bufs=1, space="PSUM") as psA0:
                qT_sb = qload.tile([128, KT, Q], F32R, name="qT_sb")
                nc.sync.dma_start(out=qT_sb,
                                  in_=qT.rearrange("(t p) m -> p t m", p=128))
                for qc in range(2):
                    psqA = psA0.tile([128, 512], F32, name="psqA", tag=f"psqA{qc}")
                    psqB = psA0.tile([64, 512], F32, name="psqB", tag=f"psqB{qc}")
                    for t in range(KT):
                        nc.tensor.matmul(psqA, wq_sb[:, t, 0:128],
                                         qT_sb[:, t, 512 * qc:512 * qc + 512],
                                         start=(t == 0), stop=(t == KT - 1))
                        nc.tensor.matmul(psqB, wq_sb[:, t, 128:192],
                                         qT_sb[:, t, 512 * qc:512 * qc + 512],
                                         start=(t == 0), stop=(t == KT - 1))
                    sl = slice(512 * qc, 512 * qc + 512)
                    nc.vector.tensor_scalar_add(qTp[0][0:64, sl], psqA[0:64, :],
                                                bqs_sb[:, 0:1])
                    nc.vector.tensor_scalar_add(qTp[1][0:64, sl], psqA[64:128, :],
                                                bqs_sb[:, 1:2])
                    nc.vector.tensor_scalar_add(qTp[2][0:64, sl], psqB[0:64, :],
                                                bqs_sb[:, 2:3])
                for h in range(HPC):
                    nc.sync.dma_start(out=qTp[h][64:96, :], in_=b1t[h, :, :])

            # ---- phase B: k/v projections, streamed over 512-chunks ----
            for ch in range(NCH):
                kvc = stream.tile([128, KT, 512], F32R, name="kvc", tag="kvc")
                nc.sync.dma_start(
                    out=kvc,
                    in_=kvT[:, 512 * ch:512 * ch + 512]
                        .rearrange("(t p) n -> p t n", p=128))
                pskA = psB.tile([128, 512], F32, name="pskA", tag="psb")
                for t in range(KT):
                    nc.tensor.matmul(pskA, wk_sb[:, t, 0:128], kvc[:, t, :],
                                     start=(t == 0), stop=(t == KT - 1))
                nc.vector.tensor_scalar_add(kbTp[0][ch][0:64, :], pskA[0:64, :],
                                            bks_sb[:, 0:1])
                nc.vector.tensor_scalar_add(kbTp[1][ch][0:64, :], pskA[64:128, :],
                                            bks_sb[:, 1:2])
                pskB = psB.tile([64, 512], F32, name="pskB", tag="psb")
                for t in range(KT):
                    nc.tensor.matmul(pskB, wk_sb[:, t, 128:192], kvc[:, t, :],
                                     start=(t == 0), stop=(t == KT - 1))
                nc.vector.tensor_scalar_add(kbTp[2][ch][0:64, :], pskB[0:64, :],
                                            bks_sb[:, 2:3])
                for h in range(HPC):
                    nc.sync.dma_start(out=kbTp[h][ch][64:96, :],
                                      in_=w_n[:, 512 * ch:512 * ch + 512])
                for s in range(4):
                    n128 = 4 * ch + s
                    psv = psB.tile([128, 256], F32, name="psv", tag="psb")
                    for t in range(KT):
                        nc.tensor.matmul(psv, kvc[:, t, 128 * s:128 * s + 128],
                                         wv_sb[:, t, :],
                                         start=(t == 0), stop=(t == KT - 1))
                    vt = vb[n128]
                    nc.vector.tensor_tensor(
                        out=vt[:, 0:195].rearrange("p (h e) -> p h e", e=65)[:, :, 0:64],
                        in0=psv[:, 0:192].rearrange("p (h d) -> p h d", d=64),
                        in1=bvb_sb.rearrange("p (h d) -> p h d", d=64),
                        op=ADD)
                    ones_dst = bass.AP(tensor=vt.tensor, offset=vt.offset + 64,
                                       ap=[list(vt.ap[0]), [65, HPC]])
                    nc.sync.dma_start(out=ones_dst, in_=ones3[:, :])

            # ---- phase C: attention, per q-half ----
            with tc.tile_pool(name="psC", bufs=1, space="PSUM") as psC, \
                 tc.tile_pool(name="pexp", bufs=3) as pexp, \
                 tc.tile_pool(name="tailp", bufs=2) as tailp:
                for qh in range(2):
                    qsl = slice(512 * qh, 512 * qh + 512)
                    with tc.tile_pool(name=f"psO{qh}", bufs=1,
                                      space="PSUM") as psO:
                        po = [psO.tile([65, 512], F32, name=f"po{h}",
                                       tag=f"po{h}") for h in range(HPC)]
                        for ci in range(N // 128):
                            c512, s = ci // 4, ci % 4
                            ssl = slice(128 * s, 128 * s + 128)
                            psSA = psC.tile([128, 1024], F32, name="psSA",
                                            tag="psSA")
                            nc.tensor.matmul(psSA[:, 0:512],
                                             kbTp[0][c512][:, ssl],
                                             qTp[0][:, qsl],
                                             start=True, stop=True)
                            nc.tensor.matmul(psSA[:, 512:1024],
                                             kbTp[1][c512][:, ssl],
                                             qTp[1][:, qsl],
                                             start=True, stop=True)
                            psSB = psC.tile([128, 512], F32, name="psSB",
                                            tag="psSB")
                            nc.tensor.matmul(psSB, kbTp[2][c512][:, ssl],
                                             qTp[2][:, qsl],
                                             start=True, stop=True)
                            pxA = pexp.tile([128, 1024], F32R, name="pxA",
                                            tag="pxA")
                            nc.scalar.activation(out=pxA, in_=psSA, func=EXP)
                            pxB = pexp.tile([128, 512], F32R, name="pxB",
                                            tag="pxB")
                            nc.scalar.activation(out=pxB, in_=psSB, func=EXP)
                            st = (ci == 0)
                            sp = (ci == N // 128 - 1)
                            nc.tensor.matmul(po[0], vb[ci][:, 0:65],
                                             pxA[:, 0:512], start=st, stop=sp)
                            nc.tensor.matmul(po[1], vb[ci][:, 65:130],
                                             pxA[:, 512:1024], start=st, stop=sp)
                            nc.tensor.matmul(po[2], vb[ci][:, 130:195],
                                             pxB, start=st, stop=sp)
                        onT = []
                        for h in range(HPC):
                            rec = tailp.tile([1, 512], F32, name=f"rec{h}",
                                             tag="rec")
                            nc.vector.reciprocal(rec, po[h][64:65, :])
                            recb = tailp.tile([64, 512], F32, name=f"recb{h}",
                                              tag="recb")
                            nc.gpsimd.partition_broadcast(recb, rec[0:1, :])
                            ot = tailp.tile([64, 512], F32R, name=f"onT{h}",
                                            tag=f"onT{h}")
                            nc.vector.tensor_tensor(out=ot, in0=po[h][0:64, :],
                                                    in1=recb, op=MUL)
                            onT.append(ot)
                    with tc.tile_pool(name=f"psF{qh}", bufs=2,
                                      space="PSUM") as psF:
                        for qt in range(4):
                            tsl = slice(128 * qt, 128 * qt + 128)
                            for nc2 in range(2):
                                nsl = slice(384 * nc2, 384 * nc2 + 384)
                                psf = psF.tile([128, 384], F32, name="psf",
                                               tag="psf")
                                for h in range(HPC):
                                    nc.tensor.matmul(psf, onT[h][:, tsl],
                                                     wo_sb[:, h, nsl],
                                                     start=(h == 0),
                                                     stop=(h == HPC - 1))
                                osb = tailp.tile([128, 384], F32, name="osb",
                                                 tag="osb")
                                nc.vector.tensor_copy(out=osb, in_=psf)
                                nc.sync.dma_start(
                                    out=out_p[512 * qh + 128 * qt:
                                              512 * qh + 128 * qt + 128, nsl],
                                    in_=osb)
    nc.compile()
    return nc


def _get_compiled():
    global _COMPILED
    if _COMPILED is None:
        _COMPILED = _build()
    return _COMPILED


def kernel(query, key_value, Wq, bq, Wk, bk, Wv, bv, Wo, bo, rel_pos_bias):
    from concourse import bass_utils

    query = np.asarray(query, np.float32)
    key_value = np.asarray(key_value, np.float32)
    Wq = np.asarray(Wq, np.float32); bq = np.asarray(bq, np.float32)
    Wk = np.asarray(Wk, np.float32); bk = np.asarray(bk, np.float32)
    Wv = np.asarray(Wv, np.float32); bv = np.asarray(bv, np.float32)
    Wo = np.asarray(Wo, np.float32); bo = np.asarray(bo, np.float32)
    rel_pos_bias = np.asarray(rel_pos_bias, np.float32)

    b1, w_n = _host_bias_parts(rel_pos_bias)
    scale = np.float32(SCALE)

    qTs = [np.ascontiguousarray(query[b].T) for b in range(B)]
    kvTs = [np.ascontiguousarray(key_value[b].T) for b in range(B)]
    ones3 = np.ones((128, HPC), np.float32)

    in_maps = []
    for c in range(N_CORES):
        b = c // (N_CORES // B)
        h0 = (c % (N_CORES // B)) * HPC
        cols = slice(D * h0, D * h0 + D * HPC)
        wv_c = np.zeros((C, 256), np.float32)
        wv_c[:, 0:D * HPC] = Wv[:, cols]
        in_maps.append({
            "qT": qTs[b],
            "kvT": kvTs[b],
            "wq": np.ascontiguousarray(Wq[:, cols] * scale),
            "wk": np.ascontiguousarray(Wk[:, cols]),
            "wv": wv_c,
            "wo": np.ascontiguousarray(Wo[cols, :].reshape(HPC, D, C)),
            "bqs": np.ascontiguousarray((bq[cols] * scale).reshape(HPC, D).T),
            "bks": np.ascontiguousarray(bk[cols].reshape(HPC, D).T),
            "bvb": np.ascontiguousarray(
                np.broadcast_to(bv[cols][None, :], (128, D * HPC))),
            "b1t": np.ascontiguousarray(
                b1[h0:h0 + HPC].transpose(0, 2, 1)),
            "w_n": w_n,
            "ones3": ones3,
        })

    nc = _get_compiled()
    res = bass_utils.run_bass_kernel_spmd(nc, in_maps,
                                          core_ids=list(range(N_CORES)))
    out = np.zeros((B, Q, C), np.float32)
    for c in range(N_CORES):
        out[c // (N_CORES // B)] += res.results[c]["out_p"]
    out += bo[None, None, :]
    return out


# revision 3
# speedup vs baseline: 1.0398x; 1.0398x over previous
"""Cross-attention (B=2, Q=1024, N=4096, C=768, H=12, D=64) with bilinearly
interpolated relative position bias, on 8 Trainium2 NeuronCores.

Sharding: core c handles batch b = c//4 and heads 3*(c%4) .. 3*(c%4)+2
(tensor-parallel over heads, data-parallel over batch). Each core outputs, per
head, the unnormalized attention output projected through Wo_h, plus the
softmax denominators; the host divides and sums the partials and adds bo.

Device algorithm per core (all matmuls float32r, moving dim >= 256):
  qbT[h]  = (Wq_h^T @ q^T) * scale + bq          [64, 1024]   (d-major)
  kbT[h]  = Wk_h^T @ kv^T + bk                   [64, 4096]
  vb[n]   = kv @ Wv_h + bv                       [4096, 64]   (n-major)
  S^T     = [kbT; Wn]^T-contraction [qbT; B1T]   K=96 fuses the interpolated
            bias: bias[h,q,n] = sum_j B1[h,q,j] * Wn[j,n]
  E^T     = exp(S^T)            (no max-subtraction; logits are O(1))
  O^T[h]  = [vb_h | 1]^T @ E^T                   [65, 512]  row 64 = denominator
  F[h]    = O^T[h]^T-contraction Wo_h            [1024, 768]  (unnormalized)
The k/v projection chunks are interleaved into the first attention pass to
keep the PE busy enough that the HAM clock gate stays at full rate.
"""

import numpy as np

B, Q, N, C = 2, 1024, 4096, 768
H, D, REL = 12, 64, 32
SCALE = 1.0 / np.sqrt(D)
HPC = 3            # heads per core
N_CORES = 8
NCH = 8            # 512-wide n-chunks

_COMPILED = None   # cached nc across kernel() calls


def _lin_coords(n_out, n_in):
    pos = np.arange(n_out, dtype=np.float32) * np.float32((n_in - 1) / (n_out - 1))
    lo = np.clip(np.floor(pos).astype(np.int32), 0, n_in - 1)
    hi = np.clip(lo + 1, 0, n_in - 1)
    w = (pos - lo.astype(np.float32)).astype(np.float32)
    return lo, hi, w


def _host_bias_parts(rel_pos_bias):
    """B1: [H, Q, 32] q-interpolated bias; Wn: [32, N] n-interp weights."""
    lq, hq, wq = _lin_coords(Q, REL)
    ln, hn, wn = _lin_coords(N, REL)
    b1 = (rel_pos_bias[:, lq, :] * (1.0 - wq)[None, :, None]
          + rel_pos_bias[:, hq, :] * wq[None, :, None]).astype(np.float32)
    w_n = np.zeros((REL, N), np.float32)
    np.add.at(w_n, (ln, np.arange(N)), (1.0 - wn))
    np.add.at(w_n, (hn, np.arange(N)), wn)
    return b1, w_n


def _build():
    import concourse.tile as tile
    from concourse import bacc, mybir
    import concourse.bass as bass

    F32 = mybir.dt.float32
    F32R = mybir.dt.float32r
    KT = 6  # C // 128 contraction tiles

    nc = bacc.Bacc("TRN2", target_bir_lowering=False, debug=False,
                   enable_asserts=False, num_devices=N_CORES)

    qT = nc.dram_tensor("qT", [C, Q], F32R, kind="ExternalInput")
    kvT = nc.dram_tensor("kvT", [C, N], F32R, kind="ExternalInput")
    wq = nc.dram_tensor("wq", [C, 192], F32R, kind="ExternalInput")    # pre-scaled
    wk = nc.dram_tensor("wk", [C, 192], F32R, kind="ExternalInput")
    wv = nc.dram_tensor("wv", [C, 256], F32R, kind="ExternalInput")    # zero-padded
    wo = nc.dram_tensor("wo", [HPC, D, C], F32R, kind="ExternalInput")
    bqs = nc.dram_tensor("bqs", [D, HPC], F32, kind="ExternalInput")   # pre-scaled
    bks = nc.dram_tensor("bks", [D, HPC], F32, kind="ExternalInput")
    bvb = nc.dram_tensor("bvb", [128, 192], F32, kind="ExternalInput")  # bcast bv
    b1t = nc.dram_tensor("b1t", [HPC, REL, Q], F32R, kind="ExternalInput")
    w_n = nc.dram_tensor("w_n", [REL, N], F32R, kind="ExternalInput")
    ones3 = nc.dram_tensor("ones3", [128, HPC], F32R, kind="ExternalInput")
    out_p = nc.dram_tensor("out_p", [HPC, Q, C], F32, kind="ExternalOutput")
    den = nc.dram_tensor("den", [HPC, Q], F32, kind="ExternalOutput")

    EXP = mybir.ActivationFunctionType.Exp
    ADD = mybir.AluOpType.add

    with tile.TileContext(nc) as tc:
        with (
            tc.tile_pool(name="wpool", bufs=1) as wpool,
            tc.tile_pool(name="persist", bufs=1) as pers,
            tc.tile_pool(name="stream", bufs=2) as stream,
            tc.tile_pool(name="psB", bufs=2, space="PSUM") as psB,
        ):
            # ---- constants / weights ----
            wq_sb = wpool.tile([128, KT, 192], F32R, name="wq_sb")
            nc.sync.dma_start(out=wq_sb, in_=wq.rearrange("(t p) m -> p t m", p=128))
            wk_sb = wpool.tile([128, KT, 192], F32R, name="wk_sb")
            nc.sync.dma_start(out=wk_sb, in_=wk.rearrange("(t p) m -> p t m", p=128))
            wv_sb = wpool.tile([128, KT, 256], F32R, name="wv_sb")
            nc.sync.dma_start(out=wv_sb, in_=wv.rearrange("(t p) m -> p t m", p=128))
            wo_sb = wpool.tile([D, HPC, C], F32R, name="wo_sb")
            nc.sync.dma_start(out=wo_sb, in_=wo.rearrange("h p n -> p h n"))
            bqs_sb = wpool.tile([D, HPC], F32, name="bqs_sb")
            nc.sync.dma_start(out=bqs_sb, in_=bqs[:, :])
            bks_sb = wpool.tile([D, HPC], F32, name="bks_sb")
            nc.sync.dma_start(out=bks_sb, in_=bks[:, :])
            bvb_sb = wpool.tile([128, 192], F32, name="bvb_sb")
            nc.sync.dma_start(out=bvb_sb, in_=bvb[:, :])

            # ---- persistent per-head / per-chunk tiles ----
            qTp = [pers.tile([96, Q], F32R, name=f"qTp{h}", tag=f"qTp{h}")
                   for h in range(HPC)]
            kbTp = [[pers.tile([96, 512], F32R, name=f"kbT{h}_{ch}",
                               tag=f"kbT{h}_{ch}")
                     for ch in range(NCH)] for h in range(HPC)]
            vb = [pers.tile([128, 195], F32R, name=f"vb{s}", tag=f"vb{s}")
                  for s in range(N // 128)]

            # ---- phase A: q projection ----
            with tc.tile_pool(name="qload", bufs=1) as qload, \
                 tc.tile_pool(name="psA0", bufs=1, space="PSUM") as psA0:
                qT_sb = qload.tile([128, KT, Q], F32R, name="qT_sb")
                nc.sync.dma_start(out=qT_sb,
                                  in_=qT.rearrange("(t p) m -> p t m", p=128))
                for qc in range(2):
                    psqA = psA0.tile([128, 512], F32, name="psqA", tag=f"psqA{qc}")
                    psqB = psA0.tile([64, 512], F32, name="psqB", tag=f"psqB{qc}")
                    for t in range(KT):
                        nc.tensor.matmul(psqA, wq_sb[:, t, 0:128],
                                         qT_sb[:, t, 512 * qc:512 * qc + 512],
                                         start=(t == 0), stop=(t == KT - 1))
                        nc.tensor.matmul(psqB, wq_sb[:, t, 128:192],
                                         qT_sb[:, t, 512 * qc:512 * qc + 512],
                                         start=(t == 0), stop=(t == KT - 1))
                    sl = slice(512 * qc, 512 * qc + 512)
                    nc.vector.tensor_scalar_add(qTp[0][0:64, sl], psqA[0:64, :],
                                                bqs_sb[:, 0:1])
                    nc.vector.tensor_scalar_add(qTp[1][0:64, sl], psqA[64:128, :],
                                                bqs_sb[:, 1:2])
                    nc.vector.tensor_scalar_add(qTp[2][0:64, sl], psqB[0:64, :],
                                                bqs_sb[:, 2:3])
                for h in range(HPC):
                    nc.sync.dma_start(out=qTp[h][64:96, :], in_=b1t[h, :, :])

            # ---- phase B chunk: k/v projections for one 512-wide n-chunk ----
            def emit_b_chunk(ch):
                kvc = stream.tile([128, KT, 512], F32R, name="kvc", tag="kvc")
                nc.sync.dma_start(
                    out=kvc,
                    in_=kvT[:, 512 * ch:512 * ch + 512]
                        .rearrange("(t p) n -> p t n", p=128))
                pskA = psB.tile([128, 512], F32, name="pskA", tag="psb")
                for t in range(KT):
                    nc.tensor.matmul(pskA, wk_sb[:, t, 0:128], kvc[:, t, :],
                                     start=(t == 0), stop=(t == KT - 1))
                nc.vector.tensor_scalar_add(kbTp[0][ch][0:64, :], pskA[0:64, :],
                                            bks_sb[:, 0:1])
                nc.vector.tensor_scalar_add(kbTp[1][ch][0:64, :], pskA[64:128, :],
                                            bks_sb[:, 1:2])
                pskB = psB.tile([64, 512], F32, name="pskB", tag="psb")
                for t in range(KT):
                    nc.tensor.matmul(pskB, wk_sb[:, t, 128:192], kvc[:, t, :],
                                     start=(t == 0), stop=(t == KT - 1))
                nc.vector.tensor_scalar_add(kbTp[2][ch][0:64, :], pskB[0:64, :],
                                            bks_sb[:, 2:3])
                for h in range(HPC):
                    nc.sync.dma_start(out=kbTp[h][ch][64:96, :],
                                      in_=w_n[:, 512 * ch:512 * ch + 512])
                for s in range(4):
                    n128 = 4 * ch + s
                    psv = psB.tile([128, 256], F32, name="psv", tag="psb")
                    for t in range(KT):
                        nc.tensor.matmul(psv, kvc[:, t, 128 * s:128 * s + 128],
                                         wv_sb[:, t, :],
                                         start=(t == 0), stop=(t == KT - 1))
                    vt = vb[n128]
                    nc.vector.tensor_tensor(
                        out=vt[:, 0:195].rearrange("p (h e) -> p h e", e=65)[:, :, 0:64],
                        in0=psv[:, 0:192].rearrange("p (h d) -> p h d", d=64),
                        in1=bvb_sb.rearrange("p (h d) -> p h d", d=64),
                        op=ADD)
                    ones_dst = bass.AP(tensor=vt.tensor, offset=vt.offset + 64,
                                       ap=[list(vt.ap[0]), [65, HPC]])
                    nc.sync.dma_start(out=ones_dst, in_=ones3[:, :])

            # ---- phase C: attention; phase B interleaved into qh==0 ----
            with tc.tile_pool(name="psC", bufs=1, space="PSUM") as psC, \
                 tc.tile_pool(name="pexp", bufs=3) as pexp, \
                 tc.tile_pool(name="tailp", bufs=2) as tailp:
                for qh in range(2):
                    qsl = slice(512 * qh, 512 * qh + 512)
                    with tc.tile_pool(name=f"psO{qh}", bufs=1,
                                      space="PSUM") as psO:
                        po = [psO.tile([65, 512], F32, name=f"po{h}",
                                       tag=f"po{h}") for h in range(HPC)]
                        for ci in range(N // 128):
                            c512, s = ci // 4, ci % 4
                            if qh == 0 and s == 0:
                                emit_b_chunk(c512)
                            ssl = slice(128 * s, 128 * s + 128)
                            psSA = psC.tile([128, 1024], F32, name="psSA",
                                            tag="psSA")
                            nc.tensor.matmul(psSA[:, 0:512],
                                             kbTp[0][c512][:, ssl],
                                             qTp[0][:, qsl],
                                             start=True, stop=True)
                            nc.tensor.matmul(psSA[:, 512:1024],
                                             kbTp[1][c512][:, ssl],
                                             qTp[1][:, qsl],
                                             start=True, stop=True)
                            psSB = psC.tile([128, 512], F32, name="psSB",
                                            tag="psSB")
                            nc.tensor.matmul(psSB, kbTp[2][c512][:, ssl],
                                             qTp[2][:, qsl],
                                             start=True, stop=True)
                            pxA = pexp.tile([128, 1024], F32R, name="pxA",
                                            tag="pxA")
                            nc.scalar.activation(out=pxA, in_=psSA, func=EXP)
                            pxB = pexp.tile([128, 512], F32R, name="pxB",
                                            tag="pxB")
                            nc.scalar.activation(out=pxB, in_=psSB, func=EXP)
                            st = (ci == 0)
                            sp = (ci == N // 128 - 1)
                            nc.tensor.matmul(po[0], vb[ci][:, 0:65],
                                             pxA[:, 0:512], start=st, stop=sp)
                            nc.tensor.matmul(po[1], vb[ci][:, 65:130],
                                             pxA[:, 512:1024], start=st, stop=sp)
                            nc.tensor.matmul(po[2], vb[ci][:, 130:195],
                                             pxB, start=st, stop=sp)
                        # unnormalized O^T -> sbuf; denominators -> dram
                        onT = []
                        for h in range(HPC):
                            ot = tailp.tile([64, 512], F32R, name=f"onT{h}",
                                            tag=f"onT{h}")
                            nc.vector.tensor_copy(out=ot, in_=po[h][0:64, :])
                            dsb = tailp.tile([1, 512], F32, name=f"dsb{h}",
                                             tag="dsb")
                            nc.vector.tensor_copy(out=dsb, in_=po[h][64:65, :])
                            nc.sync.dma_start(out=den[h, qsl], in_=dsb)
                            onT.append(ot)
                    with tc.tile_pool(name=f"psF{qh}", bufs=2,
                                      space="PSUM") as psF:
                        for qt in range(4):
                            tsl = slice(128 * qt, 128 * qt + 128)
                            row = slice(512 * qh + 128 * qt,
                                        512 * qh + 128 * qt + 128)
                            for h in range(HPC):
                                for nc2 in range(2):
                                    nsl = slice(384 * nc2, 384 * nc2 + 384)
                                    psf = psF.tile([128, 384], F32, name="psf",
                                                   tag="psf")
                                    nc.tensor.matmul(psf, onT[h][:, tsl],
                                                     wo_sb[:, h, nsl],
                                                     start=True, stop=True)
                                    osb = tailp.tile([128, 384], F32,
                                                     name="osb", tag="osb")
                                    nc.vector.tensor_copy(out=osb, in_=psf)
                                    nc.sync.dma_start(out=out_p[h, row, nsl],
                                                      in_=osb)
    nc.compile()
    return nc


def _get_compiled():
    global _COMPILED
    if _COMPILED is None:
        _COMPILED = _build()
    return _COMPILED


def _make_in_maps(query, key_value, Wq, bq, Wk, bk, Wv, bv, Wo, rel_pos_bias):
    b1, w_n = _host_bias_parts(rel_pos_bias)
    scale = np.float32(SCALE)
    qTs = [np.ascontiguousarray(query[b].T) for b in range(B)]
    kvTs = [np.ascontiguousarray(key_value[b].T) for b in range(B)]
    ones3 = np.ones((128, HPC), np.float32)
    in_maps = []
    for c in range(N_CORES):
        b = c // (N_CORES // B)
        h0 = (c % (N_CORES // B)) * HPC
        cols = slice(D * h0, D * h0 + D * HPC)
        wv_c = np.zeros((C, 256), np.float32)
        wv_c[:, 0:D * HPC] = Wv[:, cols]
        in_maps.append({
            "qT": qTs[b],
            "kvT": kvTs[b],
            "wq": np.ascontiguousarray(Wq[:, cols] * scale),
            "wk": np.ascontiguousarray(Wk[:, cols]),
            "wv": wv_c,
            "wo": np.ascontiguousarray(Wo[cols, :].reshape(HPC, D, C)),
            "bqs": np.ascontiguousarray((bq[cols] * scale).reshape(HPC, D).T),
            "bks": np.ascontiguousarray(bk[cols].reshape(HPC, D).T),
            "bvb": np.ascontiguousarray(
                np.broadcast_to(bv[cols][None, :], (128, D * HPC))),
            "b1t": np.ascontiguousarray(b1[h0:h0 + HPC].transpose(0, 2, 1)),
            "w_n": w_n,
            "ones3": ones3,
        })
    return in_maps


def kernel(query, key_value, Wq, bq, Wk, bk, Wv, bv, Wo, bo, rel_pos_bias):
    from concourse import bass_utils

    query = np.asarray(query, np.float32)
    key_value = np.asarray(key_value, np.float32)
    Wq = np.asarray(Wq, np.float32); bq = np.asarray(bq, np.float32)
    Wk = np.asarray(Wk, np.float32); bk = np.asarray(bk, np.float32)
    Wv = np.asarray(Wv, np.float32); bv = np.asarray(bv, np.float32)
    Wo = np.asarray(Wo, np.float32); bo = np.asarray(bo, np.float32)
    rel_pos_bias = np.asarray(rel_pos_bias, np.float32)

    in_maps = _make_in_maps(query, key_value, Wq, bq, Wk, bk, Wv, bv, Wo,
                            rel_pos_bias)
    nc = _get_compiled()
    res = bass_utils.run_bass_kernel_spmd(nc, in_maps,
                                          core_ids=list(range(N_CORES)))
    out = np.zeros((B, Q, C), np.float32)
    for c in range(N_CORES):
        b = c // (N_CORES // B)
        f = res.results[c]["out_p"]          # [HPC, Q, C] unnormalized
        d = res.results[c]["den"]            # [HPC, Q]
        out[b] += (f / d[:, :, None]).sum(axis=0)
    out += bo[None, None, :]
    return out


# revision 4
# speedup vs baseline: 1.2717x; 1.2230x over previous
"""Cross-attention (B=2, Q=1024, N=4096, C=768, H=12, D=64) with bilinearly
interpolated relative position bias, on 8 Trainium2 NeuronCores.

Sharding: core c handles batch b = c//4 and heads 3*(c%4) .. 3*(c%4)+2
(tensor-parallel over heads, data-parallel over batch). Each core outputs, per
head, the unnormalized attention output projected through Wo_h, plus the
softmax denominators; the host divides, sums the partials, and adds bo.

Device algorithm per core (fp16 matmul operands, fp32 accumulation):
  qbT[h]  = (Wq_h^T @ q^T) * scale + bq          [64, 1024]   (d-major)
  kbT[h]  = Wk_h^T @ kv^T + bk                   [64, 4096]
  vb[n]   = kv @ Wv_h + bv                       [4096, 64]   (n-major)
  S^T     = [kbT; Wn]^T-contraction [qbT; B1T]   K=96 fuses the interpolated
            bias: bias[h,q,n] = sum_j B1[h,q,j] * Wn[j,n]
  E^T     = exp(S^T)            (no max-subtraction; logits are O(1))
  O^T[h]  = [vb_h | 1]^T @ E^T                   [65, 1024]  row 64 = denom
  F[h]    = O^T[h]^T-contraction Wo_h            [1024, 768]  (unnormalized)

Phase structure: one head per pass (head-outer) so the S^T PSUM tile can be
double-buffered within 8 banks; the k/v projections stream chunk-by-chunk
inside pass 0 to keep the PE dense (HAM clock gate stays at full rate).
"""

import numpy as np

B, Q, N, C = 2, 1024, 4096, 768
H, D, REL = 12, 64, 32
SCALE = 1.0 / np.sqrt(D)
HPC = 3            # heads per core
N_CORES = 8
NCH = 8            # 512-wide n-chunks

_COMPILED = None   # cached nc across kernel() calls


def _lin_coords(n_out, n_in):
    pos = np.arange(n_out, dtype=np.float32) * np.float32((n_in - 1) / (n_out - 1))
    lo = np.clip(np.floor(pos).astype(np.int32), 0, n_in - 1)
    hi = np.clip(lo + 1, 0, n_in - 1)
    w = (pos - lo.astype(np.float32)).astype(np.float32)
    return lo, hi, w


def _host_bias_parts(rel_pos_bias):
    """B1: [H, Q, 32] q-interpolated bias; Wn: [32, N] n-interp weights."""
    lq, hq, wq = _lin_coords(Q, REL)
    ln, hn, wn = _lin_coords(N, REL)
    b1 = (rel_pos_bias[:, lq, :] * (1.0 - wq)[None, :, None]
          + rel_pos_bias[:, hq, :] * wq[None, :, None]).astype(np.float32)
    w_n = np.zeros((REL, N), np.float32)
    np.add.at(w_n, (ln, np.arange(N)), (1.0 - wn))
    np.add.at(w_n, (hn, np.arange(N)), wn)
    return b1, w_n


def _build():
    import concourse.tile as tile
    from concourse import bacc, mybir
    import concourse.bass as bass

    F32 = mybir.dt.float32
    F16 = mybir.dt.float16
    KT = 6  # C // 128 contraction tiles

    nc = bacc.Bacc("TRN2", target_bir_lowering=False, debug=False,
                   enable_asserts=False, num_devices=N_CORES)

    qT = nc.dram_tensor("qT", [C, Q], F16, kind="ExternalInput")
    kvT = nc.dram_tensor("kvT", [C, N], F16, kind="ExternalInput")
    wq = nc.dram_tensor("wq", [C, 192], F16, kind="ExternalInput")    # pre-scaled
    wk = nc.dram_tensor("wk", [C, 192], F16, kind="ExternalInput")
    wv = nc.dram_tensor("wv", [C, 192], F16, kind="ExternalInput")
    wo = nc.dram_tensor("wo", [HPC, D, C], F16, kind="ExternalInput")
    bqs = nc.dram_tensor("bqs", [D, HPC], F32, kind="ExternalInput")   # pre-scaled
    bks = nc.dram_tensor("bks", [D, HPC], F32, kind="ExternalInput")
    bvb = nc.dram_tensor("bvb", [128, 192], F32, kind="ExternalInput")  # bcast bv
    b1t = nc.dram_tensor("b1t", [HPC, REL, Q], F16, kind="ExternalInput")
    w_n = nc.dram_tensor("w_n", [REL, N], F16, kind="ExternalInput")
    ones3 = nc.dram_tensor("ones3", [128, HPC], F16, kind="ExternalInput")
    out_p = nc.dram_tensor("out_p", [HPC, Q, C], F32, kind="ExternalOutput")
    den = nc.dram_tensor("den", [HPC, Q], F32, kind="ExternalOutput")

    EXP = mybir.ActivationFunctionType.Exp
    ADD = mybir.AluOpType.add

    with tile.TileContext(nc) as tc:
        with (
            tc.tile_pool(name="wpool", bufs=1) as wpool,
            tc.tile_pool(name="persist", bufs=1) as pers,
            tc.tile_pool(name="stream", bufs=2) as stream,
            tc.tile_pool(name="psS", bufs=2, space="PSUM") as psSp,
            tc.tile_pool(name="pexp", bufs=3) as pexp,
            tc.tile_pool(name="tailp", bufs=2) as tailp,
        ):
            # ---- constants / weights ----
            wq_sb = wpool.tile([128, KT, 192], F16, name="wq_sb")
            nc.sync.dma_start(out=wq_sb, in_=wq.rearrange("(t p) m -> p t m", p=128))
            wk_sb = wpool.tile([128, KT, 192], F16, name="wk_sb")
            nc.sync.dma_start(out=wk_sb, in_=wk.rearrange("(t p) m -> p t m", p=128))
            wv_sb = wpool.tile([128, KT, 192], F16, name="wv_sb")
            nc.sync.dma_start(out=wv_sb, in_=wv.rearrange("(t p) m -> p t m", p=128))
            wo_sb = wpool.tile([D, HPC, C], F16, name="wo_sb")
            nc.sync.dma_start(out=wo_sb, in_=wo.rearrange("h p n -> p h n"))
            bqs_sb = wpool.tile([D, HPC], F32, name="bqs_sb")
            nc.sync.dma_start(out=bqs_sb, in_=bqs[:, :])
            bks_sb = wpool.tile([D, HPC], F32, name="bks_sb")
            nc.sync.dma_start(out=bks_sb, in_=bks[:, :])
            bvb_sb = wpool.tile([128, 192], F32, name="bvb_sb")
            nc.sync.dma_start(out=bvb_sb, in_=bvb[:, :])

            # ---- persistent per-head / per-chunk tiles ----
            qTp = [pers.tile([96, Q], F16, name=f"qTp{h}", tag=f"qTp{h}")
                   for h in range(HPC)]
            kbTp = [[pers.tile([96, 512], F16, name=f"kbT{h}_{ch}",
                               tag=f"kbT{h}_{ch}")
                     for ch in range(NCH)] for h in range(HPC)]
            vb = [pers.tile([128, 195], F16, name=f"vb{s}", tag=f"vb{s}")
                  for s in range(N // 128)]

            # ---- phase A: q projection ----
            with tc.tile_pool(name="qload", bufs=1) as qload, \
                 tc.tile_pool(name="psA0", bufs=1, space="PSUM") as psA0:
                qT_sb = qload.tile([128, KT, Q], F16, name="qT_sb")
                nc.sync.dma_start(out=qT_sb,
                                  in_=qT.rearrange("(t p) m -> p t m", p=128))
                for qc in range(2):
                    psqA = psA0.tile([128, 512], F32, name="psqA", tag=f"psqA{qc}")
                    psqB = psA0.tile([64, 512], F32, name="psqB", tag=f"psqB{qc}")
                    for t in range(KT):
                        nc.tensor.matmul(psqA, wq_sb[:, t, 0:128],
                                         qT_sb[:, t, 512 * qc:512 * qc + 512],
                                         start=(t == 0), stop=(t == KT - 1))
                        nc.tensor.matmul(psqB, wq_sb[:, t, 128:192],
                                         qT_sb[:, t, 512 * qc:512 * qc + 512],
                                         start=(t == 0), stop=(t == KT - 1))
                    sl = slice(512 * qc, 512 * qc + 512)
                    nc.vector.tensor_scalar_add(qTp[0][0:64, sl], psqA[0:64, :],
                                                bqs_sb[:, 0:1])
                    nc.vector.tensor_scalar_add(qTp[1][0:64, sl], psqA[64:128, :],
                                                bqs_sb[:, 1:2])
                    nc.vector.tensor_scalar_add(qTp[2][0:64, sl], psqB[0:64, :],
                                                bqs_sb[:, 2:3])
                for h in range(HPC):
                    nc.sync.dma_start(out=qTp[h][64:96, :], in_=b1t[h, :, :])

            # ---- phase B chunk: k/v projections for one 512-wide n-chunk ----
            def emit_b_chunk(ch, psB):
                kvc = stream.tile([128, KT, 512], F16, name="kvc", tag="kvc")
                nc.sync.dma_start(
                    out=kvc,
                    in_=kvT[:, 512 * ch:512 * ch + 512]
                        .rearrange("(t p) n -> p t n", p=128))
                pskA = psB.tile([128, 512], F32, name="pskA", tag="psb")
                for t in range(KT):
                    nc.tensor.matmul(pskA, wk_sb[:, t, 0:128], kvc[:, t, :],
                                     start=(t == 0), stop=(t == KT - 1))
                nc.vector.tensor_scalar_add(kbTp[0][ch][0:64, :], pskA[0:64, :],
                                            bks_sb[:, 0:1])
                nc.vector.tensor_scalar_add(kbTp[1][ch][0:64, :], pskA[64:128, :],
                                            bks_sb[:, 1:2])
                pskB = psB.tile([64, 512], F32, name="pskB", tag="psb")
                for t in range(KT):
                    nc.tensor.matmul(pskB, wk_sb[:, t, 128:192], kvc[:, t, :],
                                     start=(t == 0), stop=(t == KT - 1))
                nc.vector.tensor_scalar_add(kbTp[2][ch][0:64, :], pskB[0:64, :],
                                            bks_sb[:, 2:3])
                for h in range(HPC):
                    nc.sync.dma_start(out=kbTp[h][ch][64:96, :],
                                      in_=w_n[:, 512 * ch:512 * ch + 512])
                for s in range(4):
                    n128 = 4 * ch + s
                    psv = psB.tile([128, 192], F32, name="psv", tag="psb")
                    for t in range(KT):
                        nc.tensor.matmul(psv, kvc[:, t, 128 * s:128 * s + 128],
                                         wv_sb[:, t, :],
                                         start=(t == 0), stop=(t == KT - 1))
                    vt = vb[n128]
                    nc.vector.tensor_tensor(
                        out=vt[:, 0:195].rearrange("p (h e) -> p h e", e=65)[:, :, 0:64],
                        in0=psv.rearrange("p (h d) -> p h d", d=64),
                        in1=bvb_sb.rearrange("p (h d) -> p h d", d=64),
                        op=ADD)
                    ones_dst = bass.AP(tensor=vt.tensor, offset=vt.offset + 64,
                                       ap=[list(vt.ap[0]), [65, HPC]])
                    nc.sync.dma_start(out=ones_dst, in_=ones3[:, :])

            # ---- attention passes: one head per pass ----
            import contextlib
            for h in range(HPC):
                with contextlib.ExitStack() as pstk:
                    psO = pstk.enter_context(
                        tc.tile_pool(name=f"psO{h}", bufs=1, space="PSUM"))
                    po = psO.tile([65, Q], F32, name=f"po{h}", tag="po")
                    psB = None
                    if h == 0:
                        psB = pstk.enter_context(
                            tc.tile_pool(name="psB", bufs=2, space="PSUM"))
                    for ci in range(N // 128):
                        c512, s = ci // 4, ci % 4
                        if h == 0 and s == 0:
                            emit_b_chunk(c512, psB)
                        ssl = slice(128 * s, 128 * s + 128)
                        psS = psSp.tile([128, Q], F32, name="psS", tag="psS")
                        nc.tensor.matmul(psS[:, 0:512], kbTp[h][c512][:, ssl],
                                         qTp[h][:, 0:512], start=True, stop=True)
                        nc.tensor.matmul(psS[:, 512:1024], kbTp[h][c512][:, ssl],
                                         qTp[h][:, 512:1024], start=True,
                                         stop=True)
                        px = pexp.tile([128, Q], F16, name="px", tag="px")
                        nc.scalar.activation(out=px, in_=psS, func=EXP)
                        st = (ci == 0)
                        sp = (ci == N // 128 - 1)
                        nc.tensor.matmul(po[:, 0:512], vb[ci][:, 65 * h:65 * h + 65],
                                         px[:, 0:512], start=st, stop=sp)
                        nc.tensor.matmul(po[:, 512:1024],
                                         vb[ci][:, 65 * h:65 * h + 65],
                                         px[:, 512:1024], start=st, stop=sp)
                    onT = tailp.tile([64, Q], F16, name=f"onT{h}", tag="onT")
                    nc.vector.tensor_copy(out=onT, in_=po[0:64, :])
                    dsb = tailp.tile([1, Q], F32, name=f"dsb{h}", tag="dsb")
                    nc.vector.tensor_copy(out=dsb, in_=po[64:65, :])
                    nc.sync.dma_start(out=den[h, :], in_=dsb)
                # Wo for this head (overlaps the next pass via Tile lookahead)
                with tc.tile_pool(name=f"psF{h}", bufs=2, space="PSUM") as psF:
                    for qt in range(8):
                        tsl = slice(128 * qt, 128 * qt + 128)
                        for nc2 in range(2):
                            nsl = slice(384 * nc2, 384 * nc2 + 384)
                            psf = psF.tile([128, 384], F32, name="psf", tag="psf")
                            nc.tensor.matmul(psf, onT[:, tsl], wo_sb[:, h, nsl],
                                             start=True, stop=True)
                            osb = tailp.tile([128, 384], F32, name="osb",
                                             tag="osb")
                            nc.vector.tensor_copy(out=osb, in_=psf)
                            nc.sync.dma_start(out=out_p[h, tsl, nsl], in_=osb)
    nc.compile()
    return nc


def _get_compiled():
    global _COMPILED
    if _COMPILED is None:
        _COMPILED = _build()
    return _COMPILED


def _make_in_maps(query, key_value, Wq, bq, Wk, bk, Wv, bv, Wo, rel_pos_bias):
    b1, w_n = _host_bias_parts(rel_pos_bias)
    scale = np.float32(SCALE)
    f16 = np.float16
    qTs = [np.ascontiguousarray(query[b].T).astype(f16) for b in range(B)]
    kvTs = [np.ascontiguousarray(key_value[b].T).astype(f16) for b in range(B)]
    ones3 = np.ones((128, HPC), f16)
    w_n16 = w_n.astype(f16)
    in_maps = []
    for c in range(N_CORES):
        b = c // (N_CORES // B)
        h0 = (c % (N_CORES // B)) * HPC
        cols = slice(D * h0, D * h0 + D * HPC)
        in_maps.append({
            "qT": qTs[b],
            "kvT": kvTs[b],
            "wq": (Wq[:, cols] * scale).astype(f16),
            "wk": Wk[:, cols].astype(f16),
            "wv": Wv[:, cols].astype(f16),
            "wo": np.ascontiguousarray(Wo[cols, :].reshape(HPC, D, C)).astype(f16),
            "bqs": np.ascontiguousarray((bq[cols] * scale).reshape(HPC, D).T),
            "bks": np.ascontiguousarray(bk[cols].reshape(HPC, D).T),
            "bvb": np.ascontiguousarray(
                np.broadcast_to(bv[cols][None, :], (128, D * HPC))),
            "b1t": np.ascontiguousarray(
                b1[h0:h0 + HPC].transpose(0, 2, 1)).astype(f16),
            "w_n": w_n16,
            "ones3": ones3,
        })
    return in_maps


def kernel(query, key_value, Wq, bq, Wk, bk, Wv, bv, Wo, bo, rel_pos_bias):
    from concourse import bass_utils

    query = np.asarray(query, np.float32)
    key_value = np.asarray(key_value, np.float32)
    Wq = np.asarray(Wq, np.float32); bq = np.asarray(bq, np.float32)
    Wk = np.asarray(Wk, np.float32); bk = np.asarray(bk, np.float32)
    Wv = np.asarray(Wv, np.float32); bv = np.asarray(bv, np.float32)
    Wo = np.asarray(Wo, np.float32); bo = np.asarray(bo, np.float32)
    rel_pos_bias = np.asarray(rel_pos_bias, np.float32)

    in_maps = _make_in_maps(query, key_value, Wq, bq, Wk, bk, Wv, bv, Wo,
                            rel_pos_bias)
    nc = _get_compiled()
    res = bass_utils.run_bass_kernel_spmd(nc, in_maps,
                                          core_ids=list(range(N_CORES)))
    out = np.zeros((B, Q, C), np.float32)
    for c in range(N_CORES):
        b = c // (N_CORES // B)
        f = res.results[c]["out_p"]          # [HPC, Q, C] unnormalized
        d = res.results[c]["den"]            # [HPC, Q]
        out[b] += (f / d[:, :, None]).sum(axis=0)
    out += bo[None, None, :]
    return out


# revision 5
# speedup vs baseline: 1.5241x; 1.1985x over previous
"""Cross-attention (B=2, Q=1024, N=4096, C=768, H=12, D=64) with bilinearly
interpolated relative position bias, on 8 Trainium2 NeuronCores.

Sharding: core c handles batch b = c//4 and heads 3*(c%4) .. 3*(c%4)+2
(tensor-parallel over heads, data-parallel over batch). Each core outputs, per
head, the unnormalized attention output projected through Wo_h, plus the
softmax denominators; the host divides, sums the partials, and adds bo.

Device algorithm per core (fp16 matmul operands, fp32 accumulation):
  qbT[h]  = (Wq_h^T @ q^T) * scale + bq          [64, 1024]   (d-major)
  kbT[h]  = Wk_h^T @ kv^T + bk                   [64, 4096]
  vb[n]   = kv @ Wv_h + bv                       [4096, 64]   (n-major)
  S^T     = [kbT; Wn]^T-contraction [qbT; B1T]   K=96 fuses the interpolated
            bias: bias[h,q,n] = sum_j B1[h,q,j] * Wn[j,n]
  E^T     = exp(S^T)            (no max-subtraction; logits are O(1))
  O^T[h]  = [vb_h | 1]^T @ E^T                   [65, 1024]  row 64 = denom
  F[h]    = O^T[h]^T-contraction Wo_h            [1024, 768]  (unnormalized)

Phase structure: one head per pass (head-outer) so the S^T PSUM tile can be
double-buffered within 8 banks; the k/v projections stream chunk-by-chunk
inside pass 0 to keep the PE dense (HAM clock gate stays at full rate).
"""

import numpy as np

B, Q, N, C = 2, 1024, 4096, 768
H, D, REL = 12, 64, 32
SCALE = 1.0 / np.sqrt(D)
HPC = 3            # heads per core
N_CORES = 8
NCH = 8            # 512-wide n-chunks

_COMPILED = None   # cached nc across kernel() calls


def _lin_coords(n_out, n_in):
    pos = np.arange(n_out, dtype=np.float32) * np.float32((n_in - 1) / (n_out - 1))
    lo = np.clip(np.floor(pos).astype(np.int32), 0, n_in - 1)
    hi = np.clip(lo + 1, 0, n_in - 1)
    w = (pos - lo.astype(np.float32)).astype(np.float32)
    return lo, hi, w


def _host_bias_parts(rel_pos_bias):
    """B1: [H, Q, 32] q-interpolated bias; Wn: [32, N] n-interp weights."""
    lq, hq, wq = _lin_coords(Q, REL)
    ln, hn, wn = _lin_coords(N, REL)
    b1 = (rel_pos_bias[:, lq, :] * (1.0 - wq)[None, :, None]
          + rel_pos_bias[:, hq, :] * wq[None, :, None]).astype(np.float32)
    w_n = np.zeros((REL, N), np.float32)
    np.add.at(w_n, (ln, np.arange(N)), (1.0 - wn))
    np.add.at(w_n, (hn, np.arange(N)), wn)
    return b1, w_n


def _build():
    import concourse.tile as tile
    from concourse import bacc, mybir
    import concourse.bass as bass

    F32 = mybir.dt.float32
    F16 = mybir.dt.float16
    KT = 6  # C // 128 contraction tiles

    nc = bacc.Bacc("TRN2", target_bir_lowering=False, debug=False,
                   enable_asserts=False, num_devices=N_CORES)

    qT = nc.dram_tensor("qT", [C, Q], F16, kind="ExternalInput")
    kvT = nc.dram_tensor("kvT", [C, N], F16, kind="ExternalInput")
    wq = nc.dram_tensor("wq", [C, 192], F16, kind="ExternalInput")    # pre-scaled
    wk = nc.dram_tensor("wk", [C, 192], F16, kind="ExternalInput")
    wv = nc.dram_tensor("wv", [C, 192], F16, kind="ExternalInput")
    wo = nc.dram_tensor("wo", [HPC, D, C], F16, kind="ExternalInput")
    bqs = nc.dram_tensor("bqs", [D, HPC], F32, kind="ExternalInput")   # pre-scaled
    bks = nc.dram_tensor("bks", [D, HPC], F32, kind="ExternalInput")
    bvb = nc.dram_tensor("bvb", [128, 192], F32, kind="ExternalInput")  # bcast bv
    b1t = nc.dram_tensor("b1t", [HPC, REL, Q], F16, kind="ExternalInput")
    w_n = nc.dram_tensor("w_n", [REL, N], F16, kind="ExternalInput")
    ones3 = nc.dram_tensor("ones3", [128, HPC], F16, kind="ExternalInput")
    out_p = nc.dram_tensor("out_p", [HPC, Q, C], F32, kind="ExternalOutput")
    den = nc.dram_tensor("den", [HPC, Q], F32, kind="ExternalOutput")

    EXP = mybir.ActivationFunctionType.Exp
    ADD = mybir.AluOpType.add

    with tile.TileContext(nc) as tc:
        with (
            tc.tile_pool(name="wpool", bufs=1) as wpool,
            tc.tile_pool(name="persist", bufs=1) as pers,
            tc.tile_pool(name="stream", bufs=2) as stream,
            tc.tile_pool(name="psS", bufs=2, space="PSUM") as psSp,
            tc.tile_pool(name="pexp", bufs=3) as pexp,
            tc.tile_pool(name="tailp", bufs=2) as tailp,
        ):
            # ---- constants / weights ----
            wq_sb = wpool.tile([128, KT, 192], F16, name="wq_sb")
            nc.sync.dma_start(out=wq_sb, in_=wq.rearrange("(t p) m -> p t m", p=128))
            wk_sb = wpool.tile([128, KT, 192], F16, name="wk_sb")
            nc.sync.dma_start(out=wk_sb, in_=wk.rearrange("(t p) m -> p t m", p=128))
            wv_sb = wpool.tile([128, KT, 192], F16, name="wv_sb")
            nc.sync.dma_start(out=wv_sb, in_=wv.rearrange("(t p) m -> p t m", p=128))
            wo_sb = wpool.tile([D, HPC, C], F16, name="wo_sb")
            nc.sync.dma_start(out=wo_sb, in_=wo.rearrange("h p n -> p h n"))
            bqs_sb = wpool.tile([D, HPC], F32, name="bqs_sb")
            nc.sync.dma_start(out=bqs_sb, in_=bqs[:, :])
            bks_sb = wpool.tile([D, HPC], F32, name="bks_sb")
            nc.sync.dma_start(out=bks_sb, in_=bks[:, :])
            bvb_sb = wpool.tile([128, 192], F32, name="bvb_sb")
            nc.sync.dma_start(out=bvb_sb, in_=bvb[:, :])

            # ---- persistent per-head / per-chunk tiles ----
            qTp = [pers.tile([96, Q], F16, name=f"qTp{h}", tag=f"qTp{h}")
                   for h in range(HPC)]
            kbTp = [[pers.tile([96, 512], F16, name=f"kbT{h}_{ch}",
                               tag=f"kbT{h}_{ch}")
                     for ch in range(NCH)] for h in range(HPC)]
            vb = [pers.tile([128, 195], F16, name=f"vb{s}", tag=f"vb{s}")
                  for s in range(N // 128)]

            # ---- phase A: q projection ----
            with tc.tile_pool(name="qload", bufs=1) as qload, \
                 tc.tile_pool(name="psA0", bufs=1, space="PSUM") as psA0:
                qT_sb = qload.tile([128, KT, Q], F16, name="qT_sb")
                nc.sync.dma_start(out=qT_sb,
                                  in_=qT.rearrange("(t p) m -> p t m", p=128))
                for qc in range(2):
                    psqA = psA0.tile([128, 512], F32, name="psqA", tag=f"psqA{qc}")
                    psqB = psA0.tile([64, 512], F32, name="psqB", tag=f"psqB{qc}")
                    for t in range(KT):
                        nc.tensor.matmul(psqA, wq_sb[:, t, 0:128],
                                         qT_sb[:, t, 512 * qc:512 * qc + 512],
                                         start=(t == 0), stop=(t == KT - 1))
                        nc.tensor.matmul(psqB, wq_sb[:, t, 128:192],
                                         qT_sb[:, t, 512 * qc:512 * qc + 512],
                                         start=(t == 0), stop=(t == KT - 1))
                    sl = slice(512 * qc, 512 * qc + 512)
                    nc.vector.tensor_scalar_add(qTp[0][0:64, sl], psqA[0:64, :],
                                                bqs_sb[:, 0:1])
                    nc.vector.tensor_scalar_add(qTp[1][0:64, sl], psqA[64:128, :],
                                                bqs_sb[:, 1:2])
                    nc.vector.tensor_scalar_add(qTp[2][0:64, sl], psqB[0:64, :],
                                                bqs_sb[:, 2:3])
                for h in range(HPC):
                    nc.sync.dma_start(out=qTp[h][64:96, :], in_=b1t[h, :, :])

            # ---- phase B chunk: k/v projections for one 512-wide n-chunk ----
            def emit_b_chunk(ch, psB):
                kvc = stream.tile([128, KT, 512], F16, name="kvc", tag="kvc")
                nc.sync.dma_start(
                    out=kvc,
                    in_=kvT[:, 512 * ch:512 * ch + 512]
                        .rearrange("(t p) n -> p t n", p=128))
                pskA = psB.tile([128, 512], F32, name="pskA", tag="psb")
                for t in range(KT):
                    nc.tensor.matmul(pskA, wk_sb[:, t, 0:128], kvc[:, t, :],
                                     start=(t == 0), stop=(t == KT - 1))
                nc.vector.tensor_scalar_add(kbTp[0][ch][0:64, :], pskA[0:64, :],
                                            bks_sb[:, 0:1])
                nc.vector.tensor_scalar_add(kbTp[1][ch][0:64, :], pskA[64:128, :],
                                            bks_sb[:, 1:2])
                pskB = psB.tile([64, 512], F32, name="pskB", tag="psb")
                for t in range(KT):
                    nc.tensor.matmul(pskB, wk_sb[:, t, 128:192], kvc[:, t, :],
                                     start=(t == 0), stop=(t == KT - 1))
                nc.vector.tensor_scalar_add(kbTp[2][ch][0:64, :], pskB[0:64, :],
                                            bks_sb[:, 2:3])
                for h in range(HPC):
                    nc.sync.dma_start(out=kbTp[h][ch][64:96, :],
                                      in_=w_n[:, 512 * ch:512 * ch + 512])
                for s in range(4):
                    n128 = 4 * ch + s
                    psv = psB.tile([128, 192], F32, name="psv", tag="psb")
                    for t in range(KT):
                        nc.tensor.matmul(psv, kvc[:, t, 128 * s:128 * s + 128],
                                         wv_sb[:, t, :],
                                         start=(t == 0), stop=(t == KT - 1))
                    vt = vb[n128]
                    nc.vector.tensor_tensor(
                        out=vt[:, 0:195].rearrange("p (h e) -> p h e", e=65)[:, :, 0:64],
                        in0=psv.rearrange("p (h d) -> p h d", d=64),
                        in1=bvb_sb.rearrange("p (h d) -> p h d", d=64),
                        op=ADD)
                    ones_dst = bass.AP(tensor=vt.tensor, offset=vt.offset + 64,
                                       ap=[list(vt.ap[0]), [65, HPC]])
                    nc.sync.dma_start(out=ones_dst, in_=ones3[:, :])

            # ---- attention passes: one head per pass ----
            import contextlib

            def emit_s(h, ci):
                c512, s = ci // 4, ci % 4
                ssl = slice(128 * s, 128 * s + 128)
                psS = psSp.tile([128, Q], F32, name="psS", tag="psS")
                nc.tensor.matmul(psS[:, 0:512], kbTp[h][c512][:, ssl],
                                 qTp[h][:, 0:512], start=True, stop=True)
                nc.tensor.matmul(psS[:, 512:1024], kbTp[h][c512][:, ssl],
                                 qTp[h][:, 512:1024], start=True, stop=True)
                return psS

            def emit_wo(h, onT):
                with tc.tile_pool(name=f"psF{h}", bufs=2, space="PSUM") as psF:
                    for qt in range(8):
                        tsl = slice(128 * qt, 128 * qt + 128)
                        for nc2 in range(2):
                            nsl = slice(384 * nc2, 384 * nc2 + 384)
                            psf = psF.tile([128, 384], F32, name="psf", tag="psf")
                            nc.tensor.matmul(psf, onT[:, tsl], wo_sb[:, h, nsl],
                                             start=True, stop=True)
                            osb = tailp.tile([128, 384], F32, name="osb",
                                             tag="osb")
                            nc.vector.tensor_copy(out=osb, in_=psf)
                            nc.sync.dma_start(out=out_p[h, tsl, nsl], in_=osb)

            NCI = N // 128
            pending_wo = None
            for h in range(HPC):
                with contextlib.ExitStack() as pstk:
                    psO = pstk.enter_context(
                        tc.tile_pool(name=f"psO{h}", bufs=1, space="PSUM"))
                    po = psO.tile([65, Q], F32, name=f"po{h}", tag="po")
                    psB = None
                    if h == 0:
                        psB = pstk.enter_context(
                            tc.tile_pool(name="psB", bufs=2, space="PSUM"))
                        emit_b_chunk(0, psB)
                    psS_cur = emit_s(h, 0)
                    for ci in range(NCI):
                        # run-ahead work: next chunk's projections + S matmuls
                        if ci + 1 < NCI:
                            if h == 0 and (ci + 1) % 4 == 0:
                                emit_b_chunk((ci + 1) // 4, psB)
                            psS_nxt = emit_s(h, ci + 1)
                        if ci == 8 and pending_wo is not None:
                            emit_wo(*pending_wo)
                            pending_wo = None
                        px = pexp.tile([128, Q], F16, name="px", tag="px")
                        nc.scalar.activation(out=px, in_=psS_cur, func=EXP)
                        st = (ci == 0)
                        sp = (ci == NCI - 1)
                        nc.tensor.matmul(po[:, 0:512], vb[ci][:, 65 * h:65 * h + 65],
                                         px[:, 0:512], start=st, stop=sp)
                        nc.tensor.matmul(po[:, 512:1024],
                                         vb[ci][:, 65 * h:65 * h + 65],
                                         px[:, 512:1024], start=st, stop=sp)
                        if ci + 1 < NCI:
                            psS_cur = psS_nxt
                    onT = tailp.tile([64, Q], F16, name=f"onT{h}", tag="onT")
                    nc.vector.tensor_copy(out=onT, in_=po[0:64, :])
                    dsb = tailp.tile([1, Q], F32, name=f"dsb{h}", tag="dsb")
                    nc.vector.tensor_copy(out=dsb, in_=po[64:65, :])
                    nc.sync.dma_start(out=den[h, :], in_=dsb)
                pending_wo = (h, onT)
            emit_wo(*pending_wo)
    nc.compile()
    return nc


def _get_compiled():
    global _COMPILED
    if _COMPILED is None:
        _COMPILED = _build()
    return _COMPILED


def _make_in_maps(query, key_value, Wq, bq, Wk, bk, Wv, bv, Wo, rel_pos_bias):
    b1, w_n = _host_bias_parts(rel_pos_bias)
    scale = np.float32(SCALE)
    f16 = np.float16
    qTs = [np.ascontiguousarray(query[b].T).astype(f16) for b in range(B)]
    kvTs = [np.ascontiguousarray(key_value[b].T).astype(f16) for b in range(B)]
    ones3 = np.ones((128, HPC), f16)
    w_n16 = w_n.astype(f16)
    in_maps = []
    for c in range(N_CORES):
        b = c // (N_CORES // B)
        h0 = (c % (N_CORES // B)) * HPC
        cols = slice(D * h0, D * h0 + D * HPC)
        in_maps.append({
            "qT": qTs[b],
            "kvT": kvTs[b],
            "wq": (Wq[:, cols] * scale).astype(f16),
            "wk": Wk[:, cols].astype(f16),
            "wv": Wv[:, cols].astype(f16),
            "wo": np.ascontiguousarray(Wo[cols, :].reshape(HPC, D, C)).astype(f16),
            "bqs": np.ascontiguousarray((bq[cols] * scale).reshape(HPC, D).T),
            "bks": np.ascontiguousarray(bk[cols].reshape(HPC, D).T),
            "bvb": np.ascontiguousarray(
                np.broadcast_to(bv[cols][None, :], (128, D * HPC))),
            "b1t": np.ascontiguousarray(
                b1[h0:h0 + HPC].transpose(0, 2, 1)).astype(f16),
            "w_n": w_n16,
            "ones3": ones3,
        })
    return in_maps


def kernel(query, key_value, Wq, bq, Wk, bk, Wv, bv, Wo, bo, rel_pos_bias):
    from concourse import bass_utils

    query = np.asarray(query, np.float32)
    key_value = np.asarray(key_value, np.float32)
    Wq = np.asarray(Wq, np.float32); bq = np.asarray(bq, np.float32)
    Wk = np.asarray(Wk, np.float32); bk = np.asarray(bk, np.float32)
    Wv = np.asarray(Wv, np.float32); bv = np.asarray(bv, np.float32)
    Wo = np.asarray(Wo, np.float32); bo = np.asarray(bo, np.float32)
    rel_pos_bias = np.asarray(rel_pos_bias, np.float32)

    in_maps = _make_in_maps(query, key_value, Wq, bq, Wk, bk, Wv, bv, Wo,
                            rel_pos_bias)
    nc = _get_compiled()
    res = bass_utils.run_bass_kernel_spmd(nc, in_maps,
                                          core_ids=list(range(N_CORES)))
    out = np.zeros((B, Q, C), np.float32)
    for c in range(N_CORES):
        b = c // (N_CORES // B)
        f = res.results[c]["out_p"]          # [HPC, Q, C] unnormalized
        d = res.results[c]["den"]            # [HPC, Q]
        out[b] += (f / d[:, :, None]).sum(axis=0)
    out += bo[None, None, :]
    return out


# revision 6
# speedup vs baseline: 1.7039x; 1.1180x over previous
"""Cross-attention (B=2, Q=1024, N=4096, C=768, H=12, D=64) with bilinearly
interpolated relative position bias, on 8 Trainium2 NeuronCores.

Sharding: core c handles batch b = c//4 and heads 3*(c%4) .. 3*(c%4)+2
(tensor-parallel over heads, data-parallel over batch). Each core outputs, per
head, the unnormalized attention output projected through Wo_h, plus the
softmax denominators; the host divides, sums the partials, and adds bo.

Device algorithm per core (fp16 matmul operands, fp32 accumulation):
  qbT[h]  = (Wq_h^T @ q^T) * scale + bq          [64, 1024]   (d-major)
  kbT[h]  = Wk_h^T @ kv^T + bk                   [64, 4096]
  vb[n]   = kv @ Wv_h + bv                       [4096, 64]   (n-major)
  S^T     = [kbT; Wn]^T-contraction [qbT; B1T]   K=96 fuses the interpolated
            bias: bias[h,q,n] = sum_j B1[h,q,j] * Wn[j,n]
  E^T     = exp(S^T)            (no max-subtraction; logits are O(1))
  O^T[h]  = [vb_h | 1]^T @ E^T                   [65, 1024]  row 64 = denom
  F[h]    = O^T[h]^T-contraction Wo_h            [1024, 768]  (unnormalized)

Phase structure: one head per pass (head-outer) so the S^T PSUM tile can be
double-buffered within 8 banks; the k/v projections stream chunk-by-chunk
inside pass 0 to keep the PE dense (HAM clock gate stays at full rate).
"""

import numpy as np

B, Q, N, C = 2, 1024, 4096, 768
H, D, REL = 12, 64, 32
SCALE = 1.0 / np.sqrt(D)
HPC = 3            # heads per core
N_CORES = 8
NCH = 8            # 512-wide n-chunks

_COMPILED = None   # cached nc across kernel() calls


def _lin_coords(n_out, n_in):
    pos = np.arange(n_out, dtype=np.float32) * np.float32((n_in - 1) / (n_out - 1))
    lo = np.clip(np.floor(pos).astype(np.int32), 0, n_in - 1)
    hi = np.clip(lo + 1, 0, n_in - 1)
    w = (pos - lo.astype(np.float32)).astype(np.float32)
    return lo, hi, w


def _host_bias_parts(rel_pos_bias):
    """B1: [H, Q, 32] q-interpolated bias; Wn: [32, N] n-interp weights."""
    lq, hq, wq = _lin_coords(Q, REL)
    ln, hn, wn = _lin_coords(N, REL)
    b1 = (rel_pos_bias[:, lq, :] * (1.0 - wq)[None, :, None]
          + rel_pos_bias[:, hq, :] * wq[None, :, None]).astype(np.float32)
    w_n = np.zeros((REL, N), np.float32)
    np.add.at(w_n, (ln, np.arange(N)), (1.0 - wn))
    np.add.at(w_n, (hn, np.arange(N)), wn)
    return b1, w_n


def _build():
    import concourse.tile as tile
    from concourse import bacc, mybir
    import concourse.bass as bass

    F32 = mybir.dt.float32
    F16 = mybir.dt.float16
    KT = 6  # C // 128 contraction tiles

    nc = bacc.Bacc("TRN2", target_bir_lowering=False, debug=False,
                   enable_asserts=False, num_devices=N_CORES)

    qT = nc.dram_tensor("qT", [C, Q], F16, kind="ExternalInput")
    kvT = nc.dram_tensor("kvT", [C, N], F16, kind="ExternalInput")
    wq = nc.dram_tensor("wq", [C, 192], F16, kind="ExternalInput")    # pre-scaled
    wk = nc.dram_tensor("wk", [C, 192], F16, kind="ExternalInput")
    wv = nc.dram_tensor("wv", [C, 192], F16, kind="ExternalInput")
    wo = nc.dram_tensor("wo", [HPC, D, C], F16, kind="ExternalInput")
    bqs = nc.dram_tensor("bqs", [D, HPC], F32, kind="ExternalInput")   # pre-scaled
    bks = nc.dram_tensor("bks", [D, HPC], F32, kind="ExternalInput")
    bvb = nc.dram_tensor("bvb", [128, 192], F32, kind="ExternalInput")  # bcast bv
    b1t = nc.dram_tensor("b1t", [HPC, REL, Q], F16, kind="ExternalInput")
    w_n = nc.dram_tensor("w_n", [REL, N], F16, kind="ExternalInput")
    ones3 = nc.dram_tensor("ones3", [128, HPC], F16, kind="ExternalInput")
    out_p = nc.dram_tensor("out_p", [HPC, Q, C], F32, kind="ExternalOutput")
    den = nc.dram_tensor("den", [HPC, Q], F32, kind="ExternalOutput")

    EXP = mybir.ActivationFunctionType.Exp
    ADD = mybir.AluOpType.add

    with tile.TileContext(nc) as tc:
        with (
            tc.tile_pool(name="wpool", bufs=1) as wpool,
            tc.tile_pool(name="persist", bufs=1) as pers,
            tc.tile_pool(name="stream", bufs=2) as stream,
            tc.tile_pool(name="psS", bufs=2, space="PSUM") as psSp,
            tc.tile_pool(name="pexp", bufs=3) as pexp,
            tc.tile_pool(name="tailp", bufs=2) as tailp,
        ):
            # ---- constants / weights ----
            wq_sb = wpool.tile([128, KT, 192], F16, name="wq_sb")
            nc.sync.dma_start(out=wq_sb, in_=wq.rearrange("(t p) m -> p t m", p=128))
            wk_sb = wpool.tile([128, KT, 192], F16, name="wk_sb")
            nc.sync.dma_start(out=wk_sb, in_=wk.rearrange("(t p) m -> p t m", p=128))
            wv_sb = wpool.tile([128, KT, 192], F16, name="wv_sb")
            nc.sync.dma_start(out=wv_sb, in_=wv.rearrange("(t p) m -> p t m", p=128))
            wo_sb = wpool.tile([D, HPC, C], F16, name="wo_sb")
            nc.sync.dma_start(out=wo_sb, in_=wo.rearrange("h p n -> p h n"))
            bqs_sb = wpool.tile([D, HPC], F32, name="bqs_sb")
            nc.sync.dma_start(out=bqs_sb, in_=bqs[:, :])
            bks_sb = wpool.tile([D, HPC], F32, name="bks_sb")
            nc.sync.dma_start(out=bks_sb, in_=bks[:, :])
            bvb_sb = wpool.tile([128, 192], F32, name="bvb_sb")
            nc.sync.dma_start(out=bvb_sb, in_=bvb[:, :])

            # ---- persistent per-head / per-chunk tiles ----
            qTp = [pers.tile([96, Q], F16, name=f"qTp{h}", tag=f"qTp{h}")
                   for h in range(HPC)]
            kbTp = [[pers.tile([96, 512], F16, name=f"kbT{h}_{ch}",
                               tag=f"kbT{h}_{ch}")
                     for ch in range(NCH)] for h in range(HPC)]
            vb = [pers.tile([128, 195], F16, name=f"vb{s}", tag=f"vb{s}")
                  for s in range(N // 128)]

            # ---- phase A: q projection ----
            with tc.tile_pool(name="qload", bufs=1) as qload, \
                 tc.tile_pool(name="psA0", bufs=1, space="PSUM") as psA0:
                qT_sb = qload.tile([128, KT, Q], F16, name="qT_sb")
                nc.sync.dma_start(out=qT_sb,
                                  in_=qT.rearrange("(t p) m -> p t m", p=128))
                for qc in range(2):
                    psqA = psA0.tile([128, 512], F32, name="psqA", tag=f"psqA{qc}")
                    psqB = psA0.tile([64, 512], F32, name="psqB", tag=f"psqB{qc}")
                    for t in range(KT):
                        nc.tensor.matmul(psqA, wq_sb[:, t, 0:128],
                                         qT_sb[:, t, 512 * qc:512 * qc + 512],
                                         start=(t == 0), stop=(t == KT - 1))
                        nc.tensor.matmul(psqB, wq_sb[:, t, 128:192],
                                         qT_sb[:, t, 512 * qc:512 * qc + 512],
                                         start=(t == 0), stop=(t == KT - 1))
                    sl = slice(512 * qc, 512 * qc + 512)
                    nc.vector.tensor_scalar_add(qTp[0][0:64, sl], psqA[0:64, :],
                                                bqs_sb[:, 0:1])
                    nc.vector.tensor_scalar_add(qTp[1][0:64, sl], psqA[64:128, :],
                                                bqs_sb[:, 1:2])
                    nc.vector.tensor_scalar_add(qTp[2][0:64, sl], psqB[0:64, :],
                                                bqs_sb[:, 2:3])
                for h in range(HPC):
                    nc.sync.dma_start(out=qTp[h][64:96, :], in_=b1t[h, :, :])

            # ---- phase B chunk: k/v projections for one 512-wide n-chunk ----
            def emit_b_chunk(ch, psB):
                kvc = stream.tile([128, KT, 512], F16, name="kvc", tag="kvc")
                nc.sync.dma_start(
                    out=kvc,
                    in_=kvT[:, 512 * ch:512 * ch + 512]
                        .rearrange("(t p) n -> p t n", p=128))
                pskA = psB.tile([128, 512], F32, name="pskA", tag="psb")
                for t in range(KT):
                    nc.tensor.matmul(pskA, wk_sb[:, t, 0:128], kvc[:, t, :],
                                     start=(t == 0), stop=(t == KT - 1))
                nc.vector.tensor_scalar_add(kbTp[0][ch][0:64, :], pskA[0:64, :],
                                            bks_sb[:, 0:1])
                nc.vector.tensor_scalar_add(kbTp[1][ch][0:64, :], pskA[64:128, :],
                                            bks_sb[:, 1:2])
                pskB = psB.tile([64, 512], F32, name="pskB", tag="psb")
                for t in range(KT):
                    nc.tensor.matmul(pskB, wk_sb[:, t, 128:192], kvc[:, t, :],
                                     start=(t == 0), stop=(t == KT - 1))
                nc.vector.tensor_scalar_add(kbTp[2][ch][0:64, :], pskB[0:64, :],
                                            bks_sb[:, 2:3])
                for h in range(HPC):
                    nc.sync.dma_start(out=kbTp[h][ch][64:96, :],
                                      in_=w_n[:, 512 * ch:512 * ch + 512])
                for s in range(4):
                    n128 = 4 * ch + s
                    psv = psB.tile([128, 192], F32, name="psv", tag="psb")
                    for t in range(KT):
                        nc.tensor.matmul(psv, kvc[:, t, 128 * s:128 * s + 128],
                                         wv_sb[:, t, :],
                                         start=(t == 0), stop=(t == KT - 1))
                    vt = vb[n128]
                    nc.vector.tensor_tensor(
                        out=vt[:, 0:195].rearrange("p (h e) -> p h e", e=65)[:, :, 0:64],
                        in0=psv.rearrange("p (h d) -> p h d", d=64),
                        in1=bvb_sb.rearrange("p (h d) -> p h d", d=64),
                        op=ADD)
                    ones_dst = bass.AP(tensor=vt.tensor, offset=vt.offset + 64,
                                       ap=[list(vt.ap[0]), [65, HPC]])
                    nc.sync.dma_start(out=ones_dst, in_=ones3[:, :])

            # ---- attention passes: one head per pass ----
            import contextlib

            def emit_s(h, ci):
                c512, s = ci // 4, ci % 4
                ssl = slice(128 * s, 128 * s + 128)
                psS = psSp.tile([128, Q], F32, name="psS", tag="psS")
                nc.tensor.matmul(psS[:, 0:512], kbTp[h][c512][:, ssl],
                                 qTp[h][:, 0:512], start=True, stop=True)
                nc.tensor.matmul(psS[:, 512:1024], kbTp[h][c512][:, ssl],
                                 qTp[h][:, 512:1024], start=True, stop=True)
                return psS

            def emit_wo(h, onT, bufs=2):
                with tc.tile_pool(name=f"psF{h}", bufs=bufs, space="PSUM") as psF:
                    for qt in range(8):
                        tsl = slice(128 * qt, 128 * qt + 128)
                        for nc2 in range(2):
                            nsl = slice(384 * nc2, 384 * nc2 + 384)
                            psf = psF.tile([128, 384], F32, name="psf", tag="psf")
                            nc.tensor.matmul(psf, onT[:, tsl], wo_sb[:, h, nsl],
                                             start=True, stop=True)
                            osb = tailp.tile([128, 384], F32, name="osb",
                                             tag="osb", bufs=4)
                            nc.vector.tensor_copy(out=osb, in_=psf)
                            nc.sync.dma_start(out=out_p[h, tsl, nsl], in_=osb)

            NCI = N // 128
            pending_wo = None
            for h in range(HPC):
                with contextlib.ExitStack() as pstk:
                    psO = pstk.enter_context(
                        tc.tile_pool(name=f"psO{h}", bufs=1, space="PSUM"))
                    po = psO.tile([65, Q], F32, name=f"po{h}", tag="po")
                    psB = None
                    if h == 0:
                        psB = pstk.enter_context(
                            tc.tile_pool(name="psB", bufs=2, space="PSUM"))
                        emit_b_chunk(0, psB)
                    psS_cur = emit_s(h, 0)
                    for ci in range(NCI):
                        # run-ahead work: next chunk's projections + S matmuls
                        if ci + 1 < NCI:
                            if h == 0 and (ci + 1) % 4 == 0:
                                emit_b_chunk((ci + 1) // 4, psB)
                            psS_nxt = emit_s(h, ci + 1)
                        if ci == 8 and pending_wo is not None:
                            emit_wo(*pending_wo)
                            pending_wo = None
                        px = pexp.tile([128, Q], F16, name="px", tag="px")
                        nc.scalar.activation(out=px, in_=psS_cur, func=EXP)
                        st = (ci == 0)
                        sp = (ci == NCI - 1)
                        nc.tensor.matmul(po[:, 0:512], vb[ci][:, 65 * h:65 * h + 65],
                                         px[:, 0:512], start=st, stop=sp)
                        nc.tensor.matmul(po[:, 512:1024],
                                         vb[ci][:, 65 * h:65 * h + 65],
                                         px[:, 512:1024], start=st, stop=sp)
                        if ci + 1 < NCI:
                            psS_cur = psS_nxt
                    onT = tailp.tile([64, Q], F16, name=f"onT{h}", tag="onT")
                    nc.vector.tensor_copy(out=onT, in_=po[0:64, :])
                    dsb = tailp.tile([1, Q], F32, name=f"dsb{h}", tag="dsb")
                    nc.vector.tensor_copy(out=dsb, in_=po[64:65, :])
                    nc.sync.dma_start(out=den[h, :], in_=dsb)
                pending_wo = (h, onT)
            emit_wo(*pending_wo, bufs=4)
    nc.compile()
    return nc


def _get_compiled():
    global _COMPILED
    if _COMPILED is None:
        _COMPILED = _build()
    return _COMPILED


def _make_in_maps(query, key_value, Wq, bq, Wk, bk, Wv, bv, Wo, rel_pos_bias):
    b1, w_n = _host_bias_parts(rel_pos_bias)
    scale = np.float32(SCALE)
    f16 = np.float16
    qTs = [np.ascontiguousarray(query[b].T).astype(f16) for b in range(B)]
    kvTs = [np.ascontiguousarray(key_value[b].T).astype(f16) for b in range(B)]
    ones3 = np.ones((128, HPC), f16)
    w_n16 = w_n.astype(f16)
    in_maps = []
    for c in range(N_CORES):
        b = c // (N_CORES // B)
        h0 = (c % (N_CORES // B)) * HPC
        cols = slice(D * h0, D * h0 + D * HPC)
        in_maps.append({
            "qT": qTs[b],
            "kvT": kvTs[b],
            "wq": (Wq[:, cols] * scale).astype(f16),
            "wk": Wk[:, cols].astype(f16),
            "wv": Wv[:, cols].astype(f16),
            "wo": np.ascontiguousarray(Wo[cols, :].reshape(HPC, D, C)).astype(f16),
            "bqs": np.ascontiguousarray((bq[cols] * scale).reshape(HPC, D).T),
            "bks": np.ascontiguousarray(bk[cols].reshape(HPC, D).T),
            "bvb": np.ascontiguousarray(
                np.broadcast_to(bv[cols][None, :], (128, D * HPC))),
            "b1t": np.ascontiguousarray(
                b1[h0:h0 + HPC].transpose(0, 2, 1)).astype(f16),
            "w_n": w_n16,
            "ones3": ones3,
        })
    return in_maps


def kernel(query, key_value, Wq, bq, Wk, bk, Wv, bv, Wo, bo, rel_pos_bias):
    from concourse import bass_utils

    query = np.asarray(query, np.float32)
    key_value = np.asarray(key_value, np.float32)
    Wq = np.asarray(Wq, np.float32); bq = np.asarray(bq, np.float32)
    Wk = np.asarray(Wk, np.float32); bk = np.asarray(bk, np.float32)
    Wv = np.asarray(Wv, np.float32); bv = np.asarray(bv, np.float32)
    Wo = np.asarray(Wo, np.float32); bo = np.asarray(bo, np.float32)
    rel_pos_bias = np.asarray(rel_pos_bias, np.float32)

    in_maps = _make_in_maps(query, key_value, Wq, bq, Wk, bk, Wv, bv, Wo,
                            rel_pos_bias)
    nc = _get_compiled()
    res = bass_utils.run_bass_kernel_spmd(nc, in_maps,
                                          core_ids=list(range(N_CORES)))
    out = np.zeros((B, Q, C), np.float32)
    for c in range(N_CORES):
        b = c // (N_CORES // B)
        f = res.results[c]["out_p"]          # [HPC, Q, C] unnormalized
        d = res.results[c]["den"]            # [HPC, Q]
        out[b] += (f / d[:, :, None]).sum(axis=0)
    out += bo[None, None, :]
    return out
